# revision 21
# baseline (speedup 1.0000x reference)
"""GCN layer (X@W -> edge gather/scale -> segment-sum by dest -> +b -> relu)
as a Bass/Tile kernel on 8 Trainium2 NeuronCores.

Strategy (SPMD, no collectives):
  - Every core redundantly computes the FULL XW table with bf16 PE
    matmuls (X is only 51MB in bf16; the GEMM is trivial on PE), writing
    it as bf16 PAIR rows: table[50176, 128] where pair-row k =
    [feats(node 2k) | feats(node 2k+1)] = 256 B, the minimum dma_gather
    element.  No AllGather, no cross-core sync at all; replacing the
    collective with the redundant GEMM removed a ~250us serial phase.
  - X^T arrives host-shuffled: within each 512-node superblock, columns
    are ordered by node%4 class so the GEMM evicts a [128 part, 256] tile
    whose partition rows are 512B-contiguous in the table (full-rate DMA
    descriptors, batched 7 superblocks per dma_start since HWDGE costs
    ~0.6us of serialized descriptor-gen per DMA).  Evictions run on the
    otherwise-idle Activation engine, keeping DVE free for S-builds.
  - Edges partitioned by destination shard (this core's 12500 nodes),
    sorted by (dest block of 128, pair bucket of 32768 pair rows -> 2
    buckets).  Source pairs are fetched with dma_gather (int16 pair
    indices) spread across all 4 SWDGE queues -- the single descriptor
    ring was the dominant hardware bottleneck.
  - Per 128-edge chunk ONE DVE tensor_scalar builds a bf16 selection
    matrix S[e, c] = val[e] * (enc[c] == 2*dest[e]+parity[e]) against a
    host-permuted constant enc = [0,2,...,254, 1,3,...,255], so
    S[:, 0:128] selects even-parity edges (dest-major) and S[:, 128:256]
    odd ones.  Two bf16 PE matmuls accumulate
    psum[128 dests, 64] += S_even^T @ G[:, :64] + S_odd^T @ G[:, 64:]
    giving multiply + segment-sum fused.  +bias and relu on eviction.

All chunk counts are padded to the max over cores so all 8 cores run the
same program (required for the PJRT SPMD launch).
"""

import math
from contextlib import ExitStack

import numpy as np
import ml_dtypes

import concourse.bacc as bacc
import concourse.mybir as mybir
import concourse.tile as tile
from concourse.bass import _add_dep_helper
from concourse.bass_utils import run_bass_kernel_spmd

BF16 = ml_dtypes.bfloat16

# Problem constants (hardcoded per contract; kernel.py must be self-contained).
N = 100000
E = 1600000
FIN = 256
FOUT = 64
NCORES = 8

P = 128                      # partitions / block size
SHARD = N // NCORES          # 12500 dest nodes per core
NBLK = math.ceil(SHARD / P)  # 98 dest blocks per core
SHARD_PAD = NBLK * P         # 12544 (X zero-padded rows)
NFULL = NCORES * SHARD_PAD   # 100352 padded nodes
PAIR_SHARD = SHARD_PAD // 2  # 6272 pair rows per core
TABLE_PAIRS = NFULL // 2     # 50176 pair rows
PAIR_W = 2 * FOUT            # 128 bf16 = 256 B per pair row
SBN = 512                    # nodes per GEMM superblock (4 classes of 128)
NSUPER = NFULL // SBN        # 196 superblocks
WIN = 32768                  # int16-addressable window (pair rows)
NBUCKET = math.ceil(TABLE_PAIRS / WIN)  # 2
SB_BLOCKS = 8                # dest blocks per super-batch
NSB = math.ceil(NBLK / SB_BLOCKS)
KH = FIN // P                # 2 contraction halves in the GEMM


def _build_plan(edge_row, edge_col, edge_vals):
    """Host-side edge partition/sort/pad.  Returns the uniform structure
    (shared across cores) + per-core staged arrays."""
    core = edge_row // SHARD
    r_local = edge_row - core * SHARD
    blk = r_local // P
    parity = (edge_col & 1).astype(np.float32)
    # encoded dest: 2*dest_in_block + source parity, matched against the
    # permuted iota constant
    destv = 2.0 * (r_local - blk * P).astype(np.float32) + parity
    src_core = edge_col // SHARD
    table_row = src_core * SHARD_PAD + (edge_col - src_core * SHARD)
    pair = table_row >> 1
    bucket = pair // WIN
    idx16 = (pair - bucket * WIN).astype(np.int16)

    # sort edges by (core, blk, bucket)
    order = np.lexsort((bucket, blk, core))
    core_s = core[order]
    blk_s = blk[order]
    bucket_s = bucket[order]
    idx16_s = idx16[order]
    dest_s = destv[order]
    val_s = edge_vals[order].astype(np.float32)

    # segment counts per (core, blk, bucket)
    seg_key = (core_s * NBLK + blk_s) * NBUCKET + bucket_s
    counts = np.bincount(seg_key, minlength=NCORES * NBLK * NBUCKET).reshape(
        NCORES, NBLK, NBUCKET
    )
    # uniform capacity (in chunks of 128 edges) per (blk, bucket): max over cores
    chunks_bb = np.ceil(counts / P).astype(np.int64).max(axis=0)  # [NBLK, NBUCKET]
    # guarantee at least one chunk per block overall (needed so PSUM gets reset)
    assert chunks_bb.sum(axis=1).min() >= 1
    cap_bb = chunks_bb * P

    # ---- static layout ----
    # stream order: (sb, bucket, blk in sb, chunk)
    sb_of_blk = np.arange(NBLK) // SB_BLOCKS
    # slot offsets for each (blk, bucket) within its (sb, bucket) stream
    slot_off = np.zeros((NBLK, NBUCKET), dtype=np.int64)
    sb_b_len = np.zeros((NSB, NBUCKET), dtype=np.int64)   # slots per (sb, bucket)
    for sb in range(NSB):
        blks = np.where(sb_of_blk == sb)[0]
        for b in range(NBUCKET):
            off = 0
            for bk in blks:
                slot_off[bk, b] = off
                off += cap_bb[bk, b]
            sb_b_len[sb, b] = off
    # global offsets: chunk columns and idx columns per (sb, bucket)
    chunk_col0 = np.zeros((NSB, NBUCKET), dtype=np.int64)
    idx_col0 = np.zeros((NSB, NBUCKET), dtype=np.int64)
    ccur = icur = 0
    for sb in range(NSB):
        for b in range(NBUCKET):
            chunk_col0[sb, b] = ccur
            idx_col0[sb, b] = icur
            ccur += sb_b_len[sb, b] // P
            icur += sb_b_len[sb, b] // 16
    CTOT = ccur   # total chunks per core
    ITOT = icur   # total idx columns per core

    # global slot index for every edge
    first_of_seg = np.zeros(NCORES * NBLK * NBUCKET + 1, dtype=np.int64)
    np.cumsum(counts.reshape(-1), out=first_of_seg[1:])
    rank = np.arange(len(core_s)) - first_of_seg[seg_key]
    slot = (
        chunk_col0[sb_of_blk[blk_s], bucket_s] * P
        + slot_off[blk_s, bucket_s]
        + rank
    )

    # ---- per-core staged arrays ----
    idx_streams = np.zeros((NCORES, CTOT * P), dtype=np.int16)
    dest_streams = np.zeros((NCORES, CTOT * P), dtype=np.float32)
    val_streams = np.zeros((NCORES, CTOT * P), dtype=np.float32)
    for c in range(NCORES):
        m = core_s == c
        idx_streams[c, slot[m]] = idx16_s[m]
        dest_streams[c, slot[m]] = dest_s[m]
        val_streams[c, slot[m]] = val_s[m]

    # dest/val DRAM layout [128, CTOT]: chunk j, partition p <- stream[j*128+p]
    dest_np = dest_streams.reshape(NCORES, CTOT, P).transpose(0, 2, 1).copy()
    val_np = val_streams.reshape(NCORES, CTOT, P).transpose(0, 2, 1).copy()

    # idx DRAM layout [128, ITOT] int16: within each (sb,b) segment of the
    # stream, idx i -> partition i%16 (replicated over the 8 groups of 16),
    # column i//16
    idx_np = np.zeros((NCORES, P, ITOT), dtype=np.int16)
    for sb in range(NSB):
        for b in range(NBUCKET):
            L = int(sb_b_len[sb, b])
            if L == 0:
                continue
            s0 = int(chunk_col0[sb, b]) * P
            i0 = int(idx_col0[sb, b])
            seg = idx_streams[:, s0:s0 + L].reshape(NCORES, L // 16, 16)
            seg = seg.transpose(0, 2, 1)  # [NCORES, 16, L//16]
            idx_np[:, :, i0:i0 + L // 16] = np.tile(seg, (1, 8, 1))

    # per-block chunk list: (bucket, j_local_in_gather, global_chunk_col)
    blk_chunks = []
    for bk in range(NBLK):
        sb = int(sb_of_blk[bk])
        lst = []
        for b in range(NBUCKET):
            nch = int(chunks_bb[bk, b])
            j0 = int(slot_off[bk, b]) // P
            c0 = int(chunk_col0[sb, b]) + j0
            for k in range(nch):
                lst.append((b, j0 + k, c0 + k))
        blk_chunks.append(lst)

    struct = dict(
        chunks_bb=chunks_bb, sb_b_len=sb_b_len, chunk_col0=chunk_col0,
        idx_col0=idx_col0, CTOT=CTOT, ITOT=ITOT, blk_chunks=blk_chunks,
        sb_of_blk=sb_of_blk,
    )
    return struct, idx_np, dest_np, val_np


_NO_SPLIT = ("InstEventSemaphore", "InstDrain", "InstCollectiveCompute",
             "InstCall", "InstUnconditionalBranch", "InstConditionalBranch")


def _split_excess_waits(nc):
    """Deterministic post-pass: TRN2 instructions tolerate very few sync
    waits (walrus rejects with 'Too many sync wait commands').  Move all but
    one semaphore wait of every ordinary instruction onto wait-only
    InstEventSemaphore instructions inserted just before it on the same
    engine."""
    import concourse.mybir as mybir

    for blk in nc.main_func.blocks:
        out = []
        for ins in blk.instructions:
            si = ins.sync_info
            tn = type(ins).__name__
            if si is None or tn in _NO_SPLIT or len(si.on_wait) <= 1:
                out.append(ins)
                continue
            waits = list(si.on_wait)
            keep, excess = waits[:1], waits[1:]
            while excess:
                batch, excess = excess[:2], excess[2:]
                ev = mybir.InstEventSemaphore(
                    name=nc.get_next_instruction_name(), ins=[], outs=[])
                ev.engine = ins.engine
                ev.sync_info = mybir.SyncInfo(on_wait=batch, on_update=[])
                out.append(ev)
            ins.sync_info = mybir.SyncInfo(
                on_wait=keep, on_update=list(si.on_update))
            out.append(ins)
        blk.instructions[:] = out


def _build_nc(struct, variant="full"):
    st = struct
    CTOT, ITOT = st["CTOT"], st["ITOT"]
    nc = bacc.Bacc("TRN2", target_bir_lowering=False, debug=False,
                   num_devices=NCORES, num_swdge_queues=4)
    f32 = mybir.dt.float32
    bf16 = mybir.dt.bfloat16
    i16 = mybir.dt.int16

    xt_full = nc.dram_tensor("xt_full", [FIN, NFULL], bf16,
                             kind="ExternalInput")
    w_in = nc.dram_tensor("w_in", [FIN, FOUT], bf16, kind="ExternalInput")
    b_rep = nc.dram_tensor("b_rep", [P, FOUT], f32, kind="ExternalInput")
    iota_in = nc.dram_tensor("iota_in", [P, 2 * P], bf16, kind="ExternalInput")
    idx_in = nc.dram_tensor("idx_in", [P, ITOT], i16, kind="ExternalInput")
    dest_in = nc.dram_tensor("dest_in", [P, CTOT], f32, kind="ExternalInput")
    val_in = nc.dram_tensor("val_in", [P, CTOT], f32, kind="ExternalInput")

    table = nc.dram_tensor("table", [TABLE_PAIRS, PAIR_W], bf16,
                           kind="Internal")
    out_sh = nc.dram_tensor("out_sh", [SHARD_PAD, FOUT], bf16,
                            kind="ExternalOutput")

    with tile.TileContext(nc) as tc, ExitStack() as ctx:
        consts = ctx.enter_context(tc.tile_pool(name="consts", bufs=1))
        gpool = ctx.enter_context(tc.tile_pool(name="gpool", bufs=2))
        spool = ctx.enter_context(tc.tile_pool(name="spool", bufs=8))
        opool = ctx.enter_context(tc.tile_pool(name="opool", bufs=4))
        xpool = ctx.enter_context(tc.tile_pool(name="xpool", bufs=3))
        pmpool = ctx.enter_context(
            tc.tile_pool(name="pmpool", bufs=2, space="PSUM"))
        popool = ctx.enter_context(
            tc.tile_pool(name="popool", bufs=6, space="PSUM"))

        iota_t = consts.tile([P, 2 * P], bf16)
        nc.sync.dma_start(out=iota_t[:], in_=iota_in[:])
        brep_t = consts.tile([P, FOUT], f32)
        nc.sync.dma_start(out=brep_t[:], in_=b_rep[:])
        w_t = []
        for h in range(KH):
            wt = consts.tile([P, FOUT], bf16, tag=f"w{h}")
            nc.sync.dma_start(out=wt[:], in_=w_in[h * P:(h + 1) * P, :])
            w_t.append(wt)
        # edge metadata resident in SBUF for the whole kernel
        dst_all = consts.tile([P, CTOT], f32, tag="dstall")
        nc.sync.dma_start(out=dst_all[:], in_=dest_in[:])
        vl_all = consts.tile([P, CTOT], f32, tag="vlall")
        nc.sync.dma_start(out=vl_all[:], in_=val_in[:])
        idx_all = consts.tile([P, ITOT], i16, tag="idxall")
        nc.sync.dma_start(out=idx_all[:], in_=idx_in[:])

        # -------- phase 1: full-table GEMM (redundant on every core) --------
        # X^T is host-shuffled: superblock s's columns are its 256 nodes
        # ordered by node%4 class.  The [64, 256]-per-superblock eviction
        # layout maps to 512B-contiguous pair-table rows; evictions for a
        # whole load group share ONE batched DMA (HWDGE serializes at
        # ~0.6us per dma_start, so per-superblock writes would dominate).
        GRP = 3584  # nodes per X^T load group (14 superblocks)
        assert NFULL % GRP == 0 and GRP % SBN == 0
        NSBG = GRP // SBN
        for g in range(NFULL // GRP):
            xts = []
            for h in range(KH):
                xt = xpool.tile([P, GRP], bf16, tag=f"xt{h}")
                nc.sync.dma_start(
                    out=xt[:], in_=xt_full[h * P:(h + 1) * P,
                                           g * GRP:(g + 1) * GRP])
                xts.append(xt)
            omg = opool.tile([P, NSBG * 4 * FOUT], bf16, tag="om")
            for c in range(NSBG):
                mm4 = pmpool.tile([P, 4 * FOUT], f32, tag="mm")
                for k in range(4):
                    for h in range(KH):
                        nc.tensor.matmul(
                            out=mm4[:, k * FOUT:(k + 1) * FOUT],
                            lhsT=xts[h][:, c * SBN + k * P:c * SBN + (k + 1) * P],
                            rhs=w_t[h][:], start=(h == 0), stop=(h == KH - 1))
                nc.scalar.activation(
                    out=omg[:, c * 4 * FOUT:(c + 1) * 4 * FOUT], in_=mm4[:],
                    func=mybir.ActivationFunctionType.Copy)
            # pair rows [g*GRP/2, (g+1)*GRP/2): partition p holds nodes
            # {4p+k} of each superblock c -> pair rows c*256 + 2p, 2p+1
            nc.sync.dma_start(
                out=table[g * (GRP // 2):(g + 1) * (GRP // 2), :].rearrange(
                    "(c p two) w -> p c (two w)", p=P, two=2),
                in_=omg[:].rearrange("p (c q) -> p c q", c=NSBG))

        # ---------------- phase 3: gather + segment-sum ----------------
        chunks_bb = st["chunks_bb"]
        sb_b_len = st["sb_b_len"]
        chunk_col0 = st["chunk_col0"]
        idx_col0 = st["idx_col0"]
        blk_chunks = st["blk_chunks"]
        sb_of_blk = st["sb_of_blk"]

        # Gathers are spread round-robin over the 4 SWDGE queues (separate
        # descriptor rings / DMA channels) -- this alone halved the measured
        # kernel time vs a single ring.  Each gather is chained to the 4th
        # previous one (its ring predecessor) so at most ~one piece's
        # descriptors are in flight per ring (ring overflow wedges the
        # device otherwise).
        gather_insts = []

        for sb in range(NSB):
            blks = [bk for bk in range(NBLK) if sb_of_blk[bk] == sb]

            gts = [None] * NBUCKET
            for b in range(NBUCKET):
                L = int(sb_b_len[sb, b])
                if L == 0:
                    continue
                nch = L // P
                icol = int(idx_col0[sb, b])
                gt = gpool.tile([P, nch * PAIR_W], bf16, tag=f"g{b}")
                r_lo = b * WIN
                r_hi = min(r_lo + WIN, TABLE_PAIRS)
                # split into two half-gathers so consumers start on the first
                # half while the second drains
                nA = (nch + 1) // 2
                for c0, c1 in ((0, nA), (nA, nch)):
                    if c1 == c0:
                        continue
                    gi = nc.gpsimd.dma_gather(
                        out_ap=gt[:, c0 * PAIR_W:c1 * PAIR_W].rearrange(
                            "p (c f) -> p c f", f=PAIR_W),
                        in_ap=table[r_lo:r_hi, :],
                        idxs_ap=idx_all[:, icol + c0 * 8:icol + c1 * 8],
                        num_idxs=(c1 - c0) * P,
                        num_idxs_reg=(c1 - c0) * P,
                        elem_size=PAIR_W,
                        single_packet=False,
                        queue_num=len(gather_insts) % 4,
                    )
                    if len(gather_insts) >= 4:
                        _add_dep_helper(gi.ins, gather_insts[-4], sync=True,
                                        reason="swdge ring throttle")
                    gather_insts.append(gi.ins)
                gts[b] = gt

            obg = opool.tile([P, len(blks) * FOUT], bf16, tag="ob")
            for bi, bk in enumerate(blks):
                po = popool.tile([P, FOUT], f32, tag="po")
                lst = blk_chunks[bk]
                nmm = 2 * len(lst)
                for k, (b, j, gcol) in enumerate(lst):
                    s_t = spool.tile([P, 2 * P], bf16, tag="s")
                    nc.vector.tensor_scalar(
                        out=s_t[:], in0=iota_t[:],
                        scalar1=dst_all[:, gcol:gcol + 1],
                        scalar2=vl_all[:, gcol:gcol + 1],
                        op0=mybir.AluOpType.is_equal,
                        op1=mybir.AluOpType.mult,
                    )
                    # even-parity half then odd-parity half
                    nc.tensor.matmul(
                        out=po[:], lhsT=s_t[:, 0:P],
                        rhs=gts[b][:, j * PAIR_W:j * PAIR_W + FOUT],
                        start=(k == 0), stop=False)
                    nc.tensor.matmul(
                        out=po[:], lhsT=s_t[:, P:2 * P],
                        rhs=gts[b][:, j * PAIR_W + FOUT:(j + 1) * PAIR_W],
                        start=False, stop=(2 * k + 2 == nmm))
                ob = obg[:, bi * FOUT:(bi + 1) * FOUT]
                nc.vector.tensor_tensor(
                    out=ob, in0=po[:], in1=brep_t[:],
                    op=mybir.AluOpType.add)
                nc.vector.tensor_scalar(
                    out=ob, in0=ob, scalar1=0.0, scalar2=None,
                    op0=mybir.AluOpType.max)
            # one batched output write per super-batch:
            # out_sh rows [blks[0]*128, (blks[-1]+1)*128)
            nc.sync.dma_start(
                out=out_sh[blks[0] * P:(blks[-1] + 1) * P, :].rearrange(
                    "(c p) w -> p c w", p=P),
                in_=obg[:].rearrange("p (c w) -> p c w", w=FOUT))

    nc.compile()
    _split_excess_waits(nc)
    return nc


def _gemm_perm():
    """Column permutation for X^T: within each 256-node superblock, order
    nodes by node%4 class (so GEMM evictions are pair-table contiguous)."""
    perm = np.empty(NFULL, dtype=np.int64)
    q = SBN // 4
    for s in range(NSUPER):
        for k in range(4):
            perm[s * SBN + k * q:s * SBN + (k + 1) * q] = (
                s * SBN + 4 * np.arange(q) + k)
    return perm


def _prepare(X, edge_row, edge_col, edge_vals, W, b):
    """Build the compiled Bass program + per-core input maps."""
    X = np.asarray(X, dtype=np.float32)
    edge_row = np.asarray(edge_row, dtype=np.int64)
    edge_col = np.asarray(edge_col, dtype=np.int64)
    edge_vals = np.asarray(edge_vals, dtype=np.float32)
    W = np.asarray(W, dtype=np.float32)
    b = np.asarray(b, dtype=np.float32)

    struct, idx_np, dest_np, val_np = _build_plan(edge_row, edge_col, edge_vals)
    nc = _build_nc(struct)

    b_rep = np.tile(b[None, :], (P, 1)).astype(np.float32)
    # permuted iota: col c -> 2c for c<128 (even half), 2(c-128)+1 (odd half)
    enc = np.concatenate([2 * np.arange(P), 2 * np.arange(P) + 1])
    iota = np.tile(enc[None, :], (P, 1)).astype(BF16)

    # full padded X^T in bf16, quad-shuffled
    x_pad = np.zeros((NFULL, FIN), dtype=np.float32)
    for c in range(NCORES):
        x_pad[c * SHARD_PAD:c * SHARD_PAD + SHARD] = X[c * SHARD:(c + 1) * SHARD]
    perm = _gemm_perm()
    xt_full = np.ascontiguousarray(x_pad[perm].T).astype(BF16)

    w_bf = W.astype(BF16)
    in_maps = []
    for c in range(NCORES):
        in_maps.append({
            "xt_full": xt_full, "w_in": w_bf, "b_rep": b_rep,
            "iota_in": iota, "idx_in": idx_np[c],
            "dest_in": dest_np[c],
            "val_in": val_np[c],
        })
    return nc, in_maps


def _assemble(results):
    return np.concatenate(
        [results[c]["out_sh"][:SHARD] for c in range(NCORES)],
        axis=0).astype(np.float32)


def kernel(X, edge_row, edge_col, edge_vals, W, b):
    nc, in_maps = _prepare(X, edge_row, edge_col, edge_vals, W, b)
    res = run_bass_kernel_spmd(nc, in_maps, core_ids=list(range(NCORES)))
    return _assemble(res.results)


# revision 33
# speedup vs baseline: 1.1993x; 1.1993x over previous
"""GCN layer (X@W -> edge gather/scale -> segment-sum by dest -> +b -> relu)
as a Bass/Tile kernel on 8 Trainium2 NeuronCores.

Strategy (SPMD, no collectives):
  - Every core redundantly computes the FULL XW table with bf16 PE
    matmuls (X is only 51MB in bf16; the GEMM is trivial on PE), writing
    it as bf16 PAIR rows: table[50176, 128] where pair-row k =
    [feats(node 2k) | feats(node 2k+1)] = 256 B, the minimum dma_gather
    element.  No AllGather, no cross-core sync at all; replacing the
    collective with the redundant GEMM removed a ~250us serial phase.
  - X^T arrives host-shuffled: within each 512-node superblock, columns
    are ordered by node%4 class so the GEMM evicts a [128 part, 256] tile
    whose partition rows are 512B-contiguous in the table (full-rate DMA
    descriptors, batched 7 superblocks per dma_start since HWDGE costs
    ~0.6us of serialized descriptor-gen per DMA).  Evictions run on the
    otherwise-idle Activation engine, keeping DVE free for S-builds.
  - Edges partitioned by destination shard (this core's 12500 nodes),
    sorted by (dest block of 128, pair bucket of 32768 pair rows -> 2
    buckets).  Source pairs are fetched with dma_gather (int16 pair
    indices) spread across all 4 SWDGE queues -- the single descriptor
    ring was the dominant hardware bottleneck.
  - Per 128-edge chunk ONE DVE tensor_scalar builds a bf16 selection
    matrix S[e, c] = val[e] * (enc[c] == 2*dest[e]+parity[e]) against a
    host-permuted constant enc = [0,2,...,254, 1,3,...,255], so
    S[:, 0:128] selects even-parity edges (dest-major) and S[:, 128:256]
    odd ones.  Two bf16 PE matmuls accumulate
    psum[128 dests, 64] += S_even^T @ G[:, :64] + S_odd^T @ G[:, 64:]
    giving multiply + segment-sum fused.  +bias and relu on eviction.

All chunk counts are padded to the max over cores so all 8 cores run the
same program (required for the PJRT SPMD launch).
"""

import math
from contextlib import ExitStack

import numpy as np
import ml_dtypes

import concourse.bacc as bacc
import concourse.mybir as mybir
import concourse.tile as tile
from concourse.bass import _add_dep_helper
from concourse.bass_utils import run_bass_kernel_spmd

BF16 = ml_dtypes.bfloat16

# Problem constants (hardcoded per contract; kernel.py must be self-contained).
N = 100000
E = 1600000
FIN = 256
FOUT = 64
NCORES = 8

P = 128                      # partitions / block size
SHARD = N // NCORES          # 12500 dest nodes per core
NBLK = math.ceil(SHARD / P)  # 98 dest blocks per core
SHARD_PAD = NBLK * P         # 12544 (X zero-padded rows)
NFULL = NCORES * SHARD_PAD   # 100352 padded nodes
PAIR_SHARD = SHARD_PAD // 2  # 6272 pair rows per core
TABLE_PAIRS = NFULL // 2     # 50176 pair rows
PAIR_W = 2 * FOUT            # 128 bf16 = 256 B per pair row
SBN = 512                    # nodes per GEMM superblock (4 classes of 128)
NSUPER = NFULL // SBN        # 196 superblocks
WIN = 32768                  # int16-addressable window (pair rows)
NBUCKET = math.ceil(TABLE_PAIRS / WIN)  # 2
SB_BLOCKS = 8                # dest blocks per super-batch
NSB = math.ceil(NBLK / SB_BLOCKS)
KH = FIN // P                # 2 contraction halves in the GEMM


def _build_plan(edge_row, edge_col, edge_vals):
    """Host-side edge partition/sort/pad.  Returns the uniform structure
    (shared across cores) + per-core staged arrays."""
    core = edge_row // SHARD
    r_local = edge_row - core * SHARD
    blk = r_local // P
    parity = (edge_col & 1).astype(np.float32)
    # encoded dest: 2*dest_in_block + source parity, matched against the
    # permuted iota constant
    destv = 2.0 * (r_local - blk * P).astype(np.float32) + parity
    src_core = edge_col // SHARD
    table_row = src_core * SHARD_PAD + (edge_col - src_core * SHARD)
    pair = table_row >> 1
    bucket = pair // WIN
    idx16 = (pair - bucket * WIN).astype(np.int16)

    # sort edges by (core, blk, bucket)
    order = np.lexsort((bucket, blk, core))
    core_s = core[order]
    blk_s = blk[order]
    bucket_s = bucket[order]
    idx16_s = idx16[order]
    dest_s = destv[order]
    val_s = edge_vals[order].astype(np.float32)

    # segment counts per (core, blk, bucket)
    seg_key = (core_s * NBLK + blk_s) * NBUCKET + bucket_s
    counts = np.bincount(seg_key, minlength=NCORES * NBLK * NBUCKET).reshape(
        NCORES, NBLK, NBUCKET
    )
    # uniform capacity (in chunks of 128 edges) per (blk, bucket): max over cores
    chunks_bb = np.ceil(counts / P).astype(np.int64).max(axis=0)  # [NBLK, NBUCKET]
    # guarantee at least one chunk per block overall (needed so PSUM gets reset)
    assert chunks_bb.sum(axis=1).min() >= 1
    cap_bb = chunks_bb * P

    # ---- static layout ----
    # stream order: (sb, bucket, blk in sb, chunk)
    sb_of_blk = np.arange(NBLK) // SB_BLOCKS
    # slot offsets for each (blk, bucket) within its (sb, bucket) stream
    slot_off = np.zeros((NBLK, NBUCKET), dtype=np.int64)
    sb_b_len = np.zeros((NSB, NBUCKET), dtype=np.int64)   # slots per (sb, bucket)
    for sb in range(NSB):
        blks = np.where(sb_of_blk == sb)[0]
        for b in range(NBUCKET):
            off = 0
            for bk in blks:
                slot_off[bk, b] = off
                off += cap_bb[bk, b]
            sb_b_len[sb, b] = off
    # global offsets: chunk columns and idx columns per (sb, bucket)
    chunk_col0 = np.zeros((NSB, NBUCKET), dtype=np.int64)
    idx_col0 = np.zeros((NSB, NBUCKET), dtype=np.int64)
    ccur = icur = 0
    for sb in range(NSB):
        for b in range(NBUCKET):
            chunk_col0[sb, b] = ccur
            idx_col0[sb, b] = icur
            ccur += sb_b_len[sb, b] // P
            icur += sb_b_len[sb, b] // 16
    CTOT = ccur   # total chunks per core
    ITOT = icur   # total idx columns per core

    # global slot index for every edge
    first_of_seg = np.zeros(NCORES * NBLK * NBUCKET + 1, dtype=np.int64)
    np.cumsum(counts.reshape(-1), out=first_of_seg[1:])
    rank = np.arange(len(core_s)) - first_of_seg[seg_key]
    slot = (
        chunk_col0[sb_of_blk[blk_s], bucket_s] * P
        + slot_off[blk_s, bucket_s]
        + rank
    )

    # ---- per-core staged arrays ----
    idx_streams = np.zeros((NCORES, CTOT * P), dtype=np.int16)
    dest_streams = np.zeros((NCORES, CTOT * P), dtype=np.float32)
    val_streams = np.zeros((NCORES, CTOT * P), dtype=np.float32)
    for c in range(NCORES):
        m = core_s == c
        idx_streams[c, slot[m]] = idx16_s[m]
        dest_streams[c, slot[m]] = dest_s[m]
        val_streams[c, slot[m]] = val_s[m]

    # dest/val DRAM layout [128, CTOT]: chunk j, partition p <- stream[j*128+p]
    dest_np = dest_streams.reshape(NCORES, CTOT, P).transpose(0, 2, 1).copy()
    val_np = val_streams.reshape(NCORES, CTOT, P).transpose(0, 2, 1).copy()

    # idx DRAM layout [128, ITOT] int16: within each (sb,b) segment of the
    # stream, idx i -> partition i%16 (replicated over the 8 groups of 16),
    # column i//16
    idx_np = np.zeros((NCORES, P, ITOT), dtype=np.int16)
    for sb in range(NSB):
        for b in range(NBUCKET):
            L = int(sb_b_len[sb, b])
            if L == 0:
                continue
            s0 = int(chunk_col0[sb, b]) * P
            i0 = int(idx_col0[sb, b])
            seg = idx_streams[:, s0:s0 + L].reshape(NCORES, L // 16, 16)
            seg = seg.transpose(0, 2, 1)  # [NCORES, 16, L//16]
            idx_np[:, :, i0:i0 + L // 16] = np.tile(seg, (1, 8, 1))

    # per-block chunk list: (bucket, j_local_in_gather, global_chunk_col)
    blk_chunks = []
    for bk in range(NBLK):
        sb = int(sb_of_blk[bk])
        lst = []
        for b in range(NBUCKET):
            nch = int(chunks_bb[bk, b])
            j0 = int(slot_off[bk, b]) // P
            c0 = int(chunk_col0[sb, b]) + j0
            for k in range(nch):
                lst.append((b, j0 + k, c0 + k))
        blk_chunks.append(lst)

    struct = dict(
        chunks_bb=chunks_bb, sb_b_len=sb_b_len, chunk_col0=chunk_col0,
        idx_col0=idx_col0, CTOT=CTOT, ITOT=ITOT, blk_chunks=blk_chunks,
        sb_of_blk=sb_of_blk,
    )
    return struct, idx_np, dest_np, val_np


_NO_SPLIT = ("InstEventSemaphore", "InstDrain", "InstCollectiveCompute",
             "InstCall", "InstUnconditionalBranch", "InstConditionalBranch")


def _split_excess_waits(nc):
    """Deterministic post-pass: TRN2 instructions tolerate very few sync
    waits (walrus rejects with 'Too many sync wait commands').  Move all but
    one semaphore wait of every ordinary instruction onto wait-only
    InstEventSemaphore instructions inserted just before it on the same
    engine."""
    import concourse.mybir as mybir

    for blk in nc.main_func.blocks:
        out = []
        for ins in blk.instructions:
            si = ins.sync_info
            tn = type(ins).__name__
            if si is None or tn in _NO_SPLIT or len(si.on_wait) <= 1:
                out.append(ins)
                continue
            waits = list(si.on_wait)
            keep, excess = waits[:1], waits[1:]
            while excess:
                batch, excess = excess[:2], excess[2:]
                ev = mybir.InstEventSemaphore(
                    name=nc.get_next_instruction_name(), ins=[], outs=[])
                ev.engine = ins.engine
                ev.sync_info = mybir.SyncInfo(on_wait=batch, on_update=[])
                out.append(ev)
            ins.sync_info = mybir.SyncInfo(
                on_wait=keep, on_update=list(si.on_update))
            out.append(ins)
        blk.instructions[:] = out


def _build_nc(struct, variant="full"):
    st = struct
    CTOT, ITOT = st["CTOT"], st["ITOT"]
    nc = bacc.Bacc("TRN2", target_bir_lowering=False, debug=False,
                   num_devices=NCORES, num_swdge_queues=4)
    f32 = mybir.dt.float32
    bf16 = mybir.dt.bfloat16
    i16 = mybir.dt.int16

    xt_full = nc.dram_tensor("xt_full", [FIN, NFULL], bf16,
                             kind="ExternalInput")
    w_in = nc.dram_tensor("w_in", [FIN, FOUT], bf16, kind="ExternalInput")
    b_rep = nc.dram_tensor("b_rep", [P, FOUT], f32, kind="ExternalInput")
    iota_in = nc.dram_tensor("iota_in", [P, 2 * P], bf16, kind="ExternalInput")
    idx_in = nc.dram_tensor("idx_in", [P, ITOT], i16, kind="ExternalInput")
    dest_in = nc.dram_tensor("dest_in", [P, CTOT], f32, kind="ExternalInput")
    val_in = nc.dram_tensor("val_in", [P, CTOT], f32, kind="ExternalInput")

    table = nc.dram_tensor("table", [TABLE_PAIRS, PAIR_W], bf16,
                           kind="Internal")
    out_sh = nc.dram_tensor("out_sh", [SHARD_PAD, FOUT], bf16,
                            kind="ExternalOutput")

    with tile.TileContext(nc) as tc, ExitStack() as ctx:
        consts = ctx.enter_context(tc.tile_pool(name="consts", bufs=1))
        gpool = ctx.enter_context(tc.tile_pool(name="gpool", bufs=2))
        spool = ctx.enter_context(tc.tile_pool(name="spool", bufs=8))
        opool = ctx.enter_context(tc.tile_pool(name="opool", bufs=4))
        xpool = ctx.enter_context(tc.tile_pool(name="xpool", bufs=3))
        pmpool = ctx.enter_context(
            tc.tile_pool(name="pmpool", bufs=2, space="PSUM"))
        popool = ctx.enter_context(
            tc.tile_pool(name="popool", bufs=6, space="PSUM"))

        iota_t = consts.tile([P, 2 * P], bf16)
        nc.sync.dma_start(out=iota_t[:], in_=iota_in[:])
        brep_t = consts.tile([P, FOUT], f32)
        nc.sync.dma_start(out=brep_t[:], in_=b_rep[:])
        w_t = []
        for h in range(KH):
            wt = consts.tile([P, FOUT], bf16, tag=f"w{h}")
            nc.sync.dma_start(out=wt[:], in_=w_in[h * P:(h + 1) * P, :])
            w_t.append(wt)
        # edge metadata resident in SBUF for the whole kernel
        dst_all = consts.tile([P, CTOT], f32, tag="dstall")
        nc.sync.dma_start(out=dst_all[:], in_=dest_in[:])
        vl_all = consts.tile([P, CTOT], f32, tag="vlall")
        nc.sync.dma_start(out=vl_all[:], in_=val_in[:])
        idx_all = consts.tile([P, ITOT], i16, tag="idxall")
        nc.sync.dma_start(out=idx_all[:], in_=idx_in[:])

        # -------- phase 1: full-table GEMM (redundant on every core) --------
        # X^T is host-shuffled: superblock s's columns are its 256 nodes
        # ordered by node%4 class.  The [64, 256]-per-superblock eviction
        # layout maps to 512B-contiguous pair-table rows; evictions for a
        # whole load group share ONE batched DMA (HWDGE serializes at
        # ~0.6us per dma_start, so per-superblock writes would dominate).
        GRP = 3584  # nodes per X^T load group (14 superblocks)
        assert NFULL % GRP == 0 and GRP % SBN == 0
        NSBG = GRP // SBN
        for g in range(NFULL // GRP):
            xts = []
            for h in range(KH):
                xt = xpool.tile([P, GRP], bf16, tag=f"xt{h}")
                nc.sync.dma_start(
                    out=xt[:], in_=xt_full[h * P:(h + 1) * P,
                                           g * GRP:(g + 1) * GRP])
                xts.append(xt)
            omg = opool.tile([P, NSBG * 4 * FOUT], bf16, tag="om")
            for c in range(NSBG):
                mm4 = pmpool.tile([P, 4 * FOUT], f32, tag="mm")
                for k in range(4):
                    for h in range(KH):
                        nc.tensor.matmul(
                            out=mm4[:, k * FOUT:(k + 1) * FOUT],
                            lhsT=xts[h][:, c * SBN + k * P:c * SBN + (k + 1) * P],
                            rhs=w_t[h][:], start=(h == 0), stop=(h == KH - 1))
                nc.scalar.activation(
                    out=omg[:, c * 4 * FOUT:(c + 1) * 4 * FOUT], in_=mm4[:],
                    func=mybir.ActivationFunctionType.Copy)
            # pair rows [g*GRP/2, (g+1)*GRP/2): partition p holds nodes
            # {4p+k} of each superblock c -> pair rows c*256 + 2p, 2p+1
            nc.sync.dma_start(
                out=table[g * (GRP // 2):(g + 1) * (GRP // 2), :].rearrange(
                    "(c p two) w -> p c (two w)", p=P, two=2),
                in_=omg[:].rearrange("p (c q) -> p c q", c=NSBG))

        # ---------------- phase 3: gather + segment-sum ----------------
        chunks_bb = st["chunks_bb"]
        sb_b_len = st["sb_b_len"]
        chunk_col0 = st["chunk_col0"]
        idx_col0 = st["idx_col0"]
        blk_chunks = st["blk_chunks"]
        sb_of_blk = st["sb_of_blk"]

        # Gathers are spread round-robin over the 4 SWDGE queues (separate
        # descriptor rings / DMA channels) -- this alone halved the measured
        # kernel time vs a single ring.  Each gather is chained to the 4th
        # previous one (its ring predecessor) so at most ~one piece's
        # descriptors are in flight per ring (ring overflow wedges the
        # device otherwise).
        gather_insts = []

        for sb in range(NSB):
            blks = [bk for bk in range(NBLK) if sb_of_blk[bk] == sb]

            gts = [None] * NBUCKET
            for b in range(NBUCKET):
                L = int(sb_b_len[sb, b])
                if L == 0:
                    continue
                nch = L // P
                icol = int(idx_col0[sb, b])
                gt = gpool.tile([P, nch * PAIR_W], bf16, tag=f"g{b}")
                r_lo = b * WIN
                r_hi = min(r_lo + WIN, TABLE_PAIRS)
                # split into two half-gathers so consumers start on the first
                # half while the second drains
                nA = (nch + 1) // 2
                for c0, c1 in ((0, nA), (nA, nch)):
                    if c1 == c0:
                        continue
                    gi = nc.gpsimd.dma_gather(
                        out_ap=gt[:, c0 * PAIR_W:c1 * PAIR_W].rearrange(
                            "p (c f) -> p c f", f=PAIR_W),
                        in_ap=table[r_lo:r_hi, :],
                        idxs_ap=idx_all[:, icol + c0 * 8:icol + c1 * 8],
                        num_idxs=(c1 - c0) * P,
                        num_idxs_reg=(c1 - c0) * P,
                        elem_size=PAIR_W,
                        single_packet=False,
                        queue_num=len(gather_insts) % 4,
                    )
                    if len(gather_insts) >= 4:
                        _add_dep_helper(gi.ins, gather_insts[-4], sync=True,
                                        reason="swdge ring throttle")
                    gather_insts.append(gi.ins)
                gts[b] = gt

            obg = opool.tile([P, len(blks) * FOUT], bf16, tag="ob")
            for bi, bk in enumerate(blks):
                po = popool.tile([P, FOUT], f32, tag="po")
                lst = blk_chunks[bk]
                nmm = 2 * len(lst)
                for k, (b, j, gcol) in enumerate(lst):
                    s_t = spool.tile([P, 2 * P], bf16, tag="s")
                    nc.vector.tensor_scalar(
                        out=s_t[:], in0=iota_t[:],
                        scalar1=dst_all[:, gcol:gcol + 1],
                        scalar2=vl_all[:, gcol:gcol + 1],
                        op0=mybir.AluOpType.is_equal,
                        op1=mybir.AluOpType.mult,
                    )
                    # even-parity half then odd-parity half
                    nc.tensor.matmul(
                        out=po[:], lhsT=s_t[:, 0:P],
                        rhs=gts[b][:, j * PAIR_W:j * PAIR_W + FOUT],
                        start=(k == 0), stop=False)
                    nc.tensor.matmul(
                        out=po[:], lhsT=s_t[:, P:2 * P],
                        rhs=gts[b][:, j * PAIR_W + FOUT:(j + 1) * PAIR_W],
                        start=False, stop=(2 * k + 2 == nmm))
                ob = obg[:, bi * FOUT:(bi + 1) * FOUT]
                nc.vector.tensor_tensor(
                    out=ob, in0=po[:], in1=brep_t[:],
                    op=mybir.AluOpType.add)
                nc.vector.tensor_scalar(
                    out=ob, in0=ob, scalar1=0.0, scalar2=None,
                    op0=mybir.AluOpType.max)
            # one batched output write per super-batch:
            # out_sh rows [blks[0]*128, (blks[-1]+1)*128)
            nc.sync.dma_start(
                out=out_sh[blks[0] * P:(blks[-1] + 1) * P, :].rearrange(
                    "(c p) w -> p c w", p=P),
                in_=obg[:].rearrange("p (c w) -> p c w", w=FOUT))

    nc.compile()
    _split_excess_waits(nc)
    return nc


def _gemm_perm():
    """Column permutation for X^T: within each 256-node superblock, order
    nodes by node%4 class (so GEMM evictions are pair-table contiguous)."""
    perm = np.empty(NFULL, dtype=np.int64)
    q = SBN // 4
    for s in range(NSUPER):
        for k in range(4):
            perm[s * SBN + k * q:s * SBN + (k + 1) * q] = (
                s * SBN + 4 * np.arange(q) + k)
    return perm


def _prepare(X, edge_row, edge_col, edge_vals, W, b):
    """Build the compiled Bass program + per-core input maps."""
    X = np.asarray(X, dtype=np.float32)
    edge_row = np.asarray(edge_row, dtype=np.int64)
    edge_col = np.asarray(edge_col, dtype=np.int64)
    edge_vals = np.asarray(edge_vals, dtype=np.float32)
    W = np.asarray(W, dtype=np.float32)
    b = np.asarray(b, dtype=np.float32)

    struct, idx_np, dest_np, val_np = _build_plan(edge_row, edge_col, edge_vals)
    nc = _build_nc(struct)

    b_rep = np.tile(b[None, :], (P, 1)).astype(np.float32)
    # permuted iota: col c -> 2c for c<128 (even half), 2(c-128)+1 (odd half)
    enc = np.concatenate([2 * np.arange(P), 2 * np.arange(P) + 1])
    iota = np.tile(enc[None, :], (P, 1)).astype(BF16)

    # full padded X^T in bf16, quad-shuffled
    x_pad = np.zeros((NFULL, FIN), dtype=np.float32)
    for c in range(NCORES):
        x_pad[c * SHARD_PAD:c * SHARD_PAD + SHARD] = X[c * SHARD:(c + 1) * SHARD]
    perm = _gemm_perm()
    xt_full = np.ascontiguousarray(x_pad[perm].T).astype(BF16)

    w_bf = W.astype(BF16)
    in_maps = []
    for c in range(NCORES):
        in_maps.append({
            "xt_full": xt_full, "w_in": w_bf, "b_rep": b_rep,
            "iota_in": iota, "idx_in": idx_np[c],
            "dest_in": dest_np[c],
            "val_in": val_np[c],
        })
    return nc, in_maps


def _assemble(results):
    return np.concatenate(
        [results[c]["out_sh"][:SHARD] for c in range(NCORES)],
        axis=0).astype(np.float32)


def kernel(X, edge_row, edge_col, edge_vals, W, b):
    nc, in_maps = _prepare(X, edge_row, edge_col, edge_vals, W, b)
    res = run_bass_kernel_spmd(nc, in_maps, core_ids=list(range(NCORES)))
    return _assemble(res.results)
